# revision 28
# baseline (speedup 1.0000x reference)
"""Trainium2 Bass kernel for nn_Downsample (depthwise 4x4 FIR, stride 2).

Strategy: data-parallel over batch (8 cores, one batch element each).
Per (b, c) slice the separable FIR downsample runs on the tensor engine
as two band-matrix matmuls with PE transposes in between:

  out1 = A_H.T @ X          # H-downsample: [h'=128, (c w)=512] per channel pair
  T    = transpose(out1)    # PE transpose -> [w, (c h')]
  out2 = A_W.T @ T          # W-downsample: [w'=128, (pair c h')=512] per 4 channels
  out  = transpose(out2)    # -> [h', (c w')], natural output layout

Compute dtype is configurable: float32r (full-rate fp32 PE path,
~2e-4 rel err) or float16 (halves the input DMA bytes, ~1e-3 rel err).
PSUM->SBUF copies alternate between the vector and scalar engines.
"""

import numpy as np

B, C, H, W = 8, 256, 256, 256
HO, WO = H // 2, W // 2
N_CORES = 8
TAPS = 4
PAD0 = 1          # (kh - factor + 1) // 2 for kh=4, factor=2
G = 16            # channels per group (DMA batching)

VARIANT = "float16"   # "float32r" or "float16"

_CACHE = {}


def _band_matrix(g, n_in, n_out):
    """A[h, h'] = g[i] at h = 2*h' - PAD0 + i, zero-padded at the edges."""
    a = np.zeros((n_in, n_out), dtype=np.float32)
    for hp in range(n_out):
        for i in range(TAPS):
            h = 2 * hp - PAD0 + i
            if 0 <= h < n_in:
                a[h, hp] = g[i]
    return a


def _build_program(variant):
    from concourse import bacc, tile
    import concourse.mybir as mybir

    R = getattr(mybir.dt, variant)
    F32 = mybir.dt.float32

    nc = bacc.Bacc("TRN2", target_bir_lowering=False, debug=False,
                   num_devices=N_CORES)
    x_d = nc.dram_tensor("x", [C, H, W], R, kind="ExternalInput").ap()
    ah_d = nc.dram_tensor("amath", [H, HO], R, kind="ExternalInput").ap()
    aw_d = nc.dram_tensor("amatw", [W, WO], R, kind="ExternalInput").ap()
    id_d = nc.dram_tensor("ident", [128, 128], R, kind="ExternalInput").ap()
    out_dt = mybir.dt.float16 if variant == "float16" else F32
    y_d = nc.dram_tensor("y", [C, HO, WO], out_dt, kind="ExternalOutput").ap()

    n_groups = C // G

    with tile.TileContext(nc) as tc:
        with tc.tile_pool(name="const", bufs=1) as const_pool, \
             tc.tile_pool(name="xin", bufs=4) as xin_pool, \
             tc.tile_pool(name="s1p", bufs=4) as s1_pool, \
             tc.tile_pool(name="ttp", bufs=3) as tt_pool, \
             tc.tile_pool(name="s2p", bufs=3) as s2_pool, \
             tc.tile_pool(name="outp", bufs=3) as out_pool, \
             tc.tile_pool(name="ps1", bufs=2, space="PSUM") as ps1_pool, \
             tc.tile_pool(name="psT", bufs=2, space="PSUM") as psT_pool, \
             tc.tile_pool(name="ps2", bufs=2, space="PSUM") as ps2_pool, \
             tc.tile_pool(name="ps3", bufs=2, space="PSUM") as ps3_pool:

            # Constants: A_H split even/odd rows (partition p holds h=2p,2p+1
            # to match the interleaved X load); A_W split in 128-row blocks
            # (to match the PE-transpose block layout); identity.
            ah_t = const_pool.tile([128, 2, HO], R)
            aw_t = const_pool.tile([128, 2, WO], R)
            id_t = const_pool.tile([128, 128], R)
            nc.sync.dma_start(out=ah_t[:], in_=ah_d.rearrange("(k p) m -> p k m", k=2))
            nc.sync.dma_start(out=aw_t[:], in_=aw_d.rearrange("(k p) m -> p k m", k=2))
            nc.sync.dma_start(out=id_t[:], in_=id_d[:])

            for gi in range(n_groups):
                c0 = gi * G
                # X halves: [h(128) partitions, c(G), w(256)]
                xh = xin_pool.tile([128, G, 2, W], R, tag="xh")
                nc.gpsimd.dma_start(
                    out=xh[:, :, 0, :],
                    in_=x_d[c0:c0 + G, 0:128, :].rearrange("c h w -> h c w"))
                nc.gpsimd.dma_start(
                    out=xh[:, :, 1, :],
                    in_=x_d[c0:c0 + G, 128:256, :].rearrange("c h w -> h c w"))

                for half in range(G // 4):
                    # t_t holds both pairs: cols = (pair, (wh, c, h'))
                    t_t = tt_pool.tile([128, 2, 4 * HO], R, tag="t_t")
                    for pp in range(2):          # channel pair within half
                        p = half * 2 + pp        # pair index within group
                        # -- stage 1: H-downsample (2 channels -> N=512)
                        ps1 = ps1_pool.tile([128, 2 * W], F32)
                        nc.tensor.matmul(ps1[:], ah_t[:, 0, :],
                                         xh[:, 2 * p:2 * p + 2, 0, :],
                                         start=True, stop=False)
                        nc.tensor.matmul(ps1[:], ah_t[:, 1, :],
                                         xh[:, 2 * p:2 * p + 2, 1, :],
                                         start=False, stop=True)
                        s1 = s1_pool.tile([128, 2 * W], R, tag="s1")
                        if pp == 0:
                            nc.vector.tensor_copy(s1[:], ps1[:])
                        else:
                            nc.scalar.copy(s1[:], ps1[:])

                        # -- PE transpose: psT cols = (wh, c, h')
                        psT = psT_pool.tile([128, 2 * W], R)
                        for wh in range(2):
                            for cc in range(2):
                                src = s1[:, cc * W + wh * 128:
                                         cc * W + wh * 128 + 128]
                                dst = psT[:, (wh * 2 + cc) * 128:
                                          (wh * 2 + cc) * 128 + 128]
                                nc.tensor.transpose(dst, src, id_t[:])
                        if pp == 0:
                            nc.scalar.copy(t_t[:, pp, :], psT[:])
                        else:
                            nc.vector.tensor_copy(t_t[:, pp, :], psT[:])

                    # -- stage 2: W-downsample, both pairs at once (N=512)
                    ps2 = ps2_pool.tile([128, 4 * HO], F32)
                    nc.tensor.matmul(ps2[:], aw_t[:, 0, :],
                                     t_t[:, :, 0:2 * HO],
                                     start=True, stop=False)
                    nc.tensor.matmul(ps2[:], aw_t[:, 1, :],
                                     t_t[:, :, 2 * HO:4 * HO],
                                     start=False, stop=True)
                    s2 = s2_pool.tile([128, 4 * HO], R, tag="s2")
                    if half == 0:
                        nc.vector.tensor_copy(s2[:], ps2[:])
                    else:
                        nc.scalar.copy(s2[:], ps2[:])

                    # -- final PE transpose back to [h', w'] per channel
                    ps3 = ps3_pool.tile([128, 4 * WO], R)
                    for ch in range(4):
                        dst = ps3[:, ch * WO:ch * WO + WO]
                        nc.tensor.transpose(
                            dst, s2[:, ch * HO:ch * HO + HO], id_t[:])

                    outt = out_pool.tile([128, 4, WO], out_dt, tag="outt")
                    if half == 0:
                        nc.scalar.copy(outt[:], ps3[:])
                    else:
                        nc.vector.tensor_copy(outt[:], ps3[:])
                    cb = c0 + half * 4
                    nc.sync.dma_start(
                        out=y_d[cb:cb + 4, :, :].rearrange("c h w -> h c w"),
                        in_=outt[:])

    nc.compile()
    return nc


def _get_program(variant=VARIANT):
    key = "nc_" + variant
    if key not in _CACHE:
        _CACHE[key] = _build_program(variant)
    return _CACHE[key]


def kernel(x, kernel):
    from concourse.bass_utils import run_bass_kernel_spmd

    x = np.asarray(x, dtype=np.float32)
    k = np.asarray(kernel, dtype=np.float32)

    # reference correlates with the flipped kernel; separable factors from
    # row/col sums (exact for normalized separable kernels)
    w = k[::-1, ::-1].astype(np.float64)
    g_h = w.sum(axis=1)
    g_w = w.sum(axis=0)
    s = w.sum()
    if not np.isclose(s, 1.0):
        g_h = g_h / np.sqrt(s)
        g_w = g_w / np.sqrt(s)
    g_h = g_h.astype(np.float32)
    g_w = g_w.astype(np.float32)

    a_h = _band_matrix(g_h, H, HO)
    a_w = _band_matrix(g_w, W, WO)
    ident = np.eye(128, dtype=np.float32)

    np_dt = np.float16 if VARIANT == "float16" else np.float32
    a_h = a_h.astype(np_dt)
    a_w = a_w.astype(np_dt)
    ident = ident.astype(np_dt)

    nc = _get_program()
    in_maps = [
        {"x": np.ascontiguousarray(x[b]).astype(np_dt), "amath": a_h,
         "amatw": a_w, "ident": ident}
        for b in range(B)
    ]
    res = run_bass_kernel_spmd(nc, in_maps, core_ids=list(range(N_CORES)))
    _CACHE["last_result"] = res
    out = np.stack([res.results[b]["y"] for b in range(B)], axis=0)
    return out.astype(np.float32)


# revision 29
# speedup vs baseline: 1.2896x; 1.2896x over previous
"""Trainium2 Bass kernel for nn_Downsample (depthwise 4x4 FIR, stride 2).

Strategy: data-parallel over batch (8 cores, one batch element each).
Per (b, c) slice the separable FIR downsample runs on the tensor engine
as two band-matrix matmuls with PE transposes in between:

  out1 = A_H.T @ X          # H-downsample: [h'=128, (c w)=512] per channel pair
  T    = transpose(out1)    # PE transpose -> [w, (c h')]
  out2 = A_W.T @ T          # W-downsample: [w'=128, (pair c h')=512] per 4 channels
  out  = transpose(out2)    # -> [h', (c w')], natural output layout

Compute dtype is configurable: float32r (full-rate fp32 PE path,
~2e-4 rel err) or float16 (halves the input DMA bytes, ~1e-3 rel err).
PSUM->SBUF copies alternate between the vector and scalar engines.
"""

import numpy as np

B, C, H, W = 8, 256, 256, 256
HO, WO = H // 2, W // 2
N_CORES = 8
TAPS = 4
PAD0 = 1          # (kh - factor + 1) // 2 for kh=4, factor=2
G = 4             # channels per group (DMA batching)

VARIANT = "float16"   # "float32r" or "float16"

_CACHE = {}


def _band_matrix(g, n_in, n_out):
    """A[h, h'] = g[i] at h = 2*h' - PAD0 + i, zero-padded at the edges."""
    a = np.zeros((n_in, n_out), dtype=np.float32)
    for hp in range(n_out):
        for i in range(TAPS):
            h = 2 * hp - PAD0 + i
            if 0 <= h < n_in:
                a[h, hp] = g[i]
    return a


def _build_program(variant):
    from concourse import bacc, tile
    import concourse.mybir as mybir

    R = getattr(mybir.dt, variant)
    F32 = mybir.dt.float32

    nc = bacc.Bacc("TRN2", target_bir_lowering=False, debug=False,
                   num_devices=N_CORES)
    x_d = nc.dram_tensor("x", [C, H, W], R, kind="ExternalInput").ap()
    ah_d = nc.dram_tensor("amath", [H, HO], R, kind="ExternalInput").ap()
    aw_d = nc.dram_tensor("amatw", [W, WO], R, kind="ExternalInput").ap()
    id_d = nc.dram_tensor("ident", [128, 128], R, kind="ExternalInput").ap()
    out_dt = mybir.dt.float16 if variant == "float16" else F32
    y_d = nc.dram_tensor("y", [C, HO, WO], out_dt, kind="ExternalOutput").ap()

    n_groups = C // G

    with tile.TileContext(nc) as tc:
        with tc.tile_pool(name="const", bufs=1) as const_pool, \
             tc.tile_pool(name="xin", bufs=4) as xin_pool, \
             tc.tile_pool(name="s1p", bufs=4) as s1_pool, \
             tc.tile_pool(name="ttp", bufs=3) as tt_pool, \
             tc.tile_pool(name="s2p", bufs=3) as s2_pool, \
             tc.tile_pool(name="outp", bufs=3) as out_pool, \
             tc.tile_pool(name="ps1", bufs=2, space="PSUM") as ps1_pool, \
             tc.tile_pool(name="psT", bufs=2, space="PSUM") as psT_pool, \
             tc.tile_pool(name="ps2", bufs=2, space="PSUM") as ps2_pool, \
             tc.tile_pool(name="ps3", bufs=2, space="PSUM") as ps3_pool:

            # Constants: A_H split even/odd rows (partition p holds h=2p,2p+1
            # to match the interleaved X load); A_W split in 128-row blocks
            # (to match the PE-transpose block layout); identity.
            ah_t = const_pool.tile([128, 2, HO], R)
            aw_t = const_pool.tile([128, 2, WO], R)
            id_t = const_pool.tile([128, 128], R)
            nc.sync.dma_start(out=ah_t[:], in_=ah_d.rearrange("(k p) m -> p k m", k=2))
            nc.sync.dma_start(out=aw_t[:], in_=aw_d.rearrange("(k p) m -> p k m", k=2))
            nc.sync.dma_start(out=id_t[:], in_=id_d[:])

            for gi in range(n_groups):
                c0 = gi * G
                # X halves: [h(128) partitions, c(G), w(256)]
                xh = xin_pool.tile([128, G, 2, W], R, tag="xh")
                nc.gpsimd.dma_start(
                    out=xh[:, :, 0, :],
                    in_=x_d[c0:c0 + G, 0:128, :].rearrange("c h w -> h c w"))
                nc.gpsimd.dma_start(
                    out=xh[:, :, 1, :],
                    in_=x_d[c0:c0 + G, 128:256, :].rearrange("c h w -> h c w"))

                for half in range(G // 4):
                    # t_t holds both pairs: cols = (pair, (wh, c, h'))
                    t_t = tt_pool.tile([128, 2, 4 * HO], R, tag="t_t")
                    for pp in range(2):          # channel pair within half
                        p = half * 2 + pp        # pair index within group
                        # -- stage 1: H-downsample (2 channels -> N=512)
                        ps1 = ps1_pool.tile([128, 2 * W], F32)
                        nc.tensor.matmul(ps1[:], ah_t[:, 0, :],
                                         xh[:, 2 * p:2 * p + 2, 0, :],
                                         start=True, stop=False)
                        nc.tensor.matmul(ps1[:], ah_t[:, 1, :],
                                         xh[:, 2 * p:2 * p + 2, 1, :],
                                         start=False, stop=True)
                        s1 = s1_pool.tile([128, 2 * W], R, tag="s1")
                        if pp == 0:
                            nc.vector.tensor_copy(s1[:], ps1[:])
                        else:
                            nc.scalar.copy(s1[:], ps1[:])

                        # -- PE transpose: psT cols = (wh, c, h')
                        psT = psT_pool.tile([128, 2 * W], R)
                        for wh in range(2):
                            for cc in range(2):
                                src = s1[:, cc * W + wh * 128:
                                         cc * W + wh * 128 + 128]
                                dst = psT[:, (wh * 2 + cc) * 128:
                                          (wh * 2 + cc) * 128 + 128]
                                nc.tensor.transpose(dst, src, id_t[:])
                        if pp == 0:
                            nc.scalar.copy(t_t[:, pp, :], psT[:])
                        else:
                            nc.vector.tensor_copy(t_t[:, pp, :], psT[:])

                    # -- stage 2: W-downsample, both pairs at once (N=512)
                    ps2 = ps2_pool.tile([128, 4 * HO], F32)
                    nc.tensor.matmul(ps2[:], aw_t[:, 0, :],
                                     t_t[:, :, 0:2 * HO],
                                     start=True, stop=False)
                    nc.tensor.matmul(ps2[:], aw_t[:, 1, :],
                                     t_t[:, :, 2 * HO:4 * HO],
                                     start=False, stop=True)
                    s2 = s2_pool.tile([128, 4 * HO], R, tag="s2")
                    if half == 0:
                        nc.vector.tensor_copy(s2[:], ps2[:])
                    else:
                        nc.scalar.copy(s2[:], ps2[:])

                    # -- final PE transpose back to [h', w'] per channel
                    ps3 = ps3_pool.tile([128, 4 * WO], R)
                    for ch in range(4):
                        dst = ps3[:, ch * WO:ch * WO + WO]
                        nc.tensor.transpose(
                            dst, s2[:, ch * HO:ch * HO + HO], id_t[:])

                    outt = out_pool.tile([128, 4, WO], out_dt, tag="outt")
                    if half == 0:
                        nc.scalar.copy(outt[:], ps3[:])
                    else:
                        nc.vector.tensor_copy(outt[:], ps3[:])
                    cb = c0 + half * 4
                    nc.sync.dma_start(
                        out=y_d[cb:cb + 4, :, :].rearrange("c h w -> h c w"),
                        in_=outt[:])

    nc.compile()
    return nc


def _get_program(variant=VARIANT):
    key = "nc_" + variant
    if key not in _CACHE:
        _CACHE[key] = _build_program(variant)
    return _CACHE[key]


def kernel(x, kernel):
    from concourse.bass_utils import run_bass_kernel_spmd

    x = np.asarray(x, dtype=np.float32)
    k = np.asarray(kernel, dtype=np.float32)

    # reference correlates with the flipped kernel; separable factors from
    # row/col sums (exact for normalized separable kernels)
    w = k[::-1, ::-1].astype(np.float64)
    g_h = w.sum(axis=1)
    g_w = w.sum(axis=0)
    s = w.sum()
    if not np.isclose(s, 1.0):
        g_h = g_h / np.sqrt(s)
        g_w = g_w / np.sqrt(s)
    g_h = g_h.astype(np.float32)
    g_w = g_w.astype(np.float32)

    a_h = _band_matrix(g_h, H, HO)
    a_w = _band_matrix(g_w, W, WO)
    ident = np.eye(128, dtype=np.float32)

    np_dt = np.float16 if VARIANT == "float16" else np.float32
    a_h = a_h.astype(np_dt)
    a_w = a_w.astype(np_dt)
    ident = ident.astype(np_dt)

    nc = _get_program()
    in_maps = [
        {"x": np.ascontiguousarray(x[b]).astype(np_dt), "amath": a_h,
         "amatw": a_w, "ident": ident}
        for b in range(B)
    ]
    res = run_bass_kernel_spmd(nc, in_maps, core_ids=list(range(N_CORES)))
    _CACHE["last_result"] = res
    out = np.stack([res.results[b]["y"] for b in range(B)], axis=0)
    return out.astype(np.float32)
